# revision 1
# baseline (speedup 1.0000x reference)
"""Causal single-head attention on 8 Trainium2 NeuronCores.

Problem: x[4, 2048, 1024] fp32, Wq/Wk/Wv[1024, 1024] fp32.
  q,k,v = x@Wq, x@Wk, x@Wv ; out = softmax(mask(q k^T)/32) @ v

Sharding (SPMD — one program, 8 cores, per-core data):
  core = 2*b + h  handles batch b, queries {t : t % 2 == h} (1024 queries).
  The interleaved (mod-2) query split makes the causal block structure
  shape-identical across cores: per-core q-block jb (256 queries, spanning
  global positions [512*jb, 512*jb+512)) needs k-tiles 0..4*(jb+1)-1 on
  every core.  Causal masking inside the 4 diagonal k-tiles depends only on
  (u = t-4*jb, parity h) => 4 mask tiles passed as per-core data.

Layouts:
  - host passes x[b]^T ([D, T]) so Q^T/K^T/V all come out of matmuls with
    no on-chip transposes (contraction over d_in needs d_in on partitions).
  - scores are computed transposed ([k, q]) so that expS can feed the
    attn @ V matmul directly as the stationary operand (lhsT = expS[k, q]).
  - softmax denominator: DVE accumulates partition-partial sums, one tiny
    fp32 ones-matmul per q-sub reduces across partitions.  No
    max-subtraction (logits/32 are ~N(0, 0.41^2); exp never overflows).
  - K^T projection is split across the core pair and exchanged with two
    pipelined pair-AllGathers (~1 MB each) that hide behind the V + Q
    projections.  V's gather would be 8 MB / ~109 us — not worth it.

Dtypes: bf16 matmul inputs for projections and scores (fp32 PSUM accum),
float32r V / expS context matmuls (full fp32 rate at N>=256), fp32 softmax.

Measured on HW: ~214-215 us exec (stable across runs), rel err 2.8e-3
(bf16-rounding floor; identical to the numpy golden model of the same
arithmetic).  PE is ~94% busy over its span; the remaining floor is ~167 us
of required matmul cycles + weight-load/dispatch overhead, the pair-gather
launch latency, ~18 us DMA/engine spin-up, and the ~12 us TileContext
drain/barrier tail.
"""

import os
import numpy as np
import ml_dtypes

import concourse.mybir as mybir
import concourse.tile as tile
from concourse import bacc

F32 = mybir.dt.float32
F32R = mybir.dt.float32r
BF16 = mybir.dt.bfloat16
BF16_NP = ml_dtypes.bfloat16

B, T, D = 4, 2048, 1024
P = 128
DC = D // P          # 8 contraction chunks
NW = T // 512        # 4 token windows (K/V projection granularity)
NT = T // P          # 16 key tiles
QB = 256             # queries per q-block (per core)
NJB = (T // 2) // QB # 4 q-blocks per core
SCALE = 1.0 / 32.0   # 1/sqrt(D)

# V / expS storage dtype: fp32 + float32r matmuls (accurate, full rate at
# N>=256).  Flip to BF16 if hardware shows fp32r matmuls are slow.
V_F32 = True
V_DT = F32R if V_F32 else BF16   # fp32r: walrus requires producers to round
MASK_NEG = -1.0e9
# Split the K^T projection across the core pair (each core projects its own
# 1024 tokens) and AllGather the halves; the 2 MB bf16 gather (~39 us) hides
# completely behind the full V + Q projections.  V stays locally projected —
# its 8 MB gather measured ~109 us and stalls the PE (tried, reverted).
K_SPLIT = True
PAIRS = [[0, 1], [2, 3], [4, 5], [6, 7]]
_EXP = mybir.ActivationFunctionType.Exp


def _emit(nc, tc, xT_d, xTk_d, xTq_d, wq_d, wk_d, wv_d, masks_d, out_d):
    HT = T // 2  # queries per core

    def mm(out, lhsT, rhs, start, stop, **kw):
        if out.dtype == F32 and lhsT.dtype == F32:
            lhsT = lhsT.bitcast(F32R)
            rhs = rhs.bitcast(F32R)
        nc.tensor.matmul(out, lhsT, rhs, start=start, stop=stop, **kw)

    with (
        tc.sbuf_pool(name="persist", bufs=1) as persist,
        tc.psum_pool(name="p512", bufs=3) as p512,
        tc.psum_pool(name="p256", bufs=4) as p256,
        tc.psum_pool(name="pden", bufs=1) as pden,
    ):
        # ---- persistent SBUF tensors (attention-only tiles are scoped to
        # the attention phase so Q-proj inputs fit alongside the KV pools) --
        K_sb = persist.tile([P, DC * T], BF16, tag="K", name="K_sb")
        V_sb = persist.tile([P, NT * D], V_DT, tag="V", name="V_sb")
        Q_sb = persist.tile([P, DC * HT], BF16, tag="Q", name="Q_sb")

        # ---- projections: K^T (pair-split + AllGather) and V (local) ----
        with (
            tc.sbuf_pool(name="wkvp", bufs=1) as wkv_pool,
            tc.sbuf_pool(name="xtwp", bufs=2) as xtw_pool,
            tc.sbuf_pool(name="stgp", bufs=6) as stg_pool,
            tc.tile_pool(name="drp", bufs=1, space="DRAM") as dr_pool,
            nc.named_scope("kv_proj"),
        ):
            wk_sb = wkv_pool.tile([P, DC * D], BF16, tag="wk", name="wk_sb")
            wv_sb = wkv_pool.tile([P, DC * D], BF16, tag="wv", name="wv_sb")
            for c in range(DC):
                nc.sync.dma_start(out=wk_sb[:, c * D:(c + 1) * D],
                                  in_=wk_d[c * P:(c + 1) * P, :])
            if K_SPLIT:
                # K^T of own token half first; one pipelined AllGather per
                # 512-token window so gather #0 launches while window 1 is
                # still projecting (pair gathers have ~20us launch latency).
                klocs, kgs = [], []
                for w in range(NW // 2):
                    klocs.append(dr_pool.tile([D, 512], BF16, tag=f"kloc{w}",
                                              name=f"kloc{w}"))
                    kgs.append(dr_pool.tile([2, D, 512], BF16, tag=f"kg{w}",
                                            name=f"kg{w}"))
                for w in range(NW // 2):
                    xtk = xtw_pool.tile([P, DC * 512], BF16, tag="xtw",
                                        name="xtk", bufs=3)
                    for c in range(DC):
                        nc.sync.dma_start(
                            out=xtk[:, c * 512:(c + 1) * 512],
                            in_=xTk_d[c * P:(c + 1) * P, 512 * w:512 * (w + 1)])
                    for c2 in range(DC):
                        ps = p512.tile([P, 512], F32, tag="mm512", name="ps_k")
                        for c in range(DC):
                            mm(ps, wk_sb[:, c * D + P * c2: c * D + P * (c2 + 1)],
                               xtk[:, c * 512:(c + 1) * 512], c == 0, c == DC - 1)
                        st = stg_pool.tile([P, 512], BF16, tag="stk", name="stk")
                        nc.scalar.copy(out=st, in_=ps)
                        nc.sync.dma_start(
                            out=klocs[w][c2 * P:(c2 + 1) * P, :], in_=st)
                    nc.gpsimd.collective_compute(
                        "AllGather", mybir.AluOpType.bypass,
                        replica_groups=PAIRS, ins=[klocs[w][:]],
                        outs=[kgs[w][:]])
                for lw in range(NW // 2):
                    for r in range(2):
                        gw = 2 * r + lw  # global token window
                        for c in range(DC):
                            nc.sync.dma_start(
                                out=K_sb[:, c * T + 512 * gw:
                                         c * T + 512 * (gw + 1)],
                                in_=kgs[lw][r, c * P:(c + 1) * P, :])
            # V (full, local) — PE work here hides the K gather
            for c in range(DC):
                nc.sync.dma_start(out=wv_sb[:, c * D:(c + 1) * D],
                                  in_=wv_d[c * P:(c + 1) * P, :])
            for w in range(NW):
                xtw = xtw_pool.tile([P, DC * 512], BF16, tag="xtw", name="xtw",
                                    bufs=3)
                for c in range(DC):
                    nc.sync.dma_start(
                        out=xtw[:, c * 512:(c + 1) * 512],
                        in_=xT_d[c * P:(c + 1) * P, 512 * w:512 * (w + 1)])
                if not K_SPLIT:
                    for c2 in range(DC):
                        ps = p512.tile([P, 512], F32, tag="mm512", name="ps_k")
                        for c in range(DC):
                            mm(ps, wk_sb[:, c * D + P * c2: c * D + P * (c2 + 1)],
                               xtw[:, c * 512:(c + 1) * 512], c == 0, c == DC - 1)
                        nc.scalar.copy(
                            out=K_sb[:, c2 * T + 512 * w: c2 * T + 512 * (w + 1)],
                            in_=ps)
                for ts in range(4):
                    t = 4 * w + ts
                    for n in range(2):
                        ps = p512.tile([P, 512], F32, tag="mm512", name="ps_v")
                        for c in range(DC):
                            mm(ps, xtw[:, c * 512 + P * ts: c * 512 + P * (ts + 1)],
                               wv_sb[:, c * D + 512 * n: c * D + 512 * (n + 1)],
                               c == 0, c == DC - 1)
                        nc.scalar.copy(
                            out=V_sb[:, t * D + 512 * n: t * D + 512 * (n + 1)],
                            in_=ps)

            # Q-proj inputs: loaded inside this scope (a later pool scope
            # would stall the DMAs on the KV pools' SBUF release) but after
            # the V-loop emission so they queue behind the sooner-needed
            # wv/xT loads.
            wq_sb = wkv_pool.tile([P, DC * D], BF16, tag="wq", name="wq_sb")
            for c in range(DC):
                nc.sync.dma_start(out=wq_sb[:, c * D:(c + 1) * D],
                                  in_=wq_d[c * P:(c + 1) * P, :])
            xtqs = []
            for jp in range(NJB // 2):
                xtq = xtw_pool.tile([P, DC * 512], BF16, tag="xtq",
                                    name="xtq")
                for c in range(DC):
                    nc.sync.dma_start(
                        out=xtq[:, c * 512:(c + 1) * 512],
                        in_=xTq_d[c * P:(c + 1) * P, 512 * jp:512 * (jp + 1)])
                xtqs.append(xtq)
            # ---- Q^T projection (own queries, two q-blocks per matmul) --
            with nc.named_scope("q_proj"):
                for jp in range(NJB // 2):
                    xtq = xtqs[jp]
                    for c2 in range(DC):
                        ps = p512.tile([P, 512], F32, tag="mm512", name="ps_q")
                        for c in range(DC):
                            mm(ps,
                               wq_sb[:, c * D + P * c2: c * D + P * (c2 + 1)],
                               xtq[:, c * 512:(c + 1) * 512], c == 0,
                               c == DC - 1)
                        nc.scalar.copy(
                            out=Q_sb[:, c2 * HT + 512 * jp:
                                     c2 * HT + 512 * (jp + 1)],
                            in_=ps)

        # ---- attention, per q-block ----
        with (
            tc.sbuf_pool(name="attnp", bufs=1) as attnp,
            tc.sbuf_pool(name="recipp", bufs=2) as recip_pool,
            tc.sbuf_pool(name="accp", bufs=2) as acc_pool,
            tc.sbuf_pool(name="outp", bufs=4) as out_pool,
            nc.named_scope("attn"),
        ):
            expS = attnp.tile([P, NT * QB], V_DT, tag="E", name="expS")
            mask_sb = attnp.tile([P, 4 * QB], F32, tag="M", name="mask_sb")
            ones_f32 = attnp.tile([P, 1], F32, tag="O32", name="ones_f32")
            nc.vector.memset(ones_f32, 1.0)
            for u in range(4):
                nc.sync.dma_start(out=mask_sb[:, u * QB:(u + 1) * QB],
                                  in_=masks_d[u])
            for jb in range(NJB):
                kt = 4 * (jb + 1)  # k-tiles needed by this q-block
                # pass 1: scores^T -> exp (-> mask on the 4 diagonal tiles)
                for t in range(kt):
                    ps = p256.tile([P, QB], F32, tag="mm256", name="ps_s")
                    for c in range(DC):
                        mm(ps, K_sb[:, c * T + P * t: c * T + P * (t + 1)],
                           Q_sb[:, c * HT + QB * jb: c * HT + QB * (jb + 1)],
                           c == 0, c == DC - 1)
                    if t >= kt - 4:
                        u = t - (kt - 4)
                        nc.vector.tensor_add(ps, ps,
                                             mask_sb[:, u * QB:(u + 1) * QB])
                    nc.scalar.activation(out=expS[:, t * QB:(t + 1) * QB], in_=ps,
                                         func=_EXP, scale=SCALE)
                # denominators: den[q, s] = sum_k expS[k, q].  Partition-
                # partial sums accumulate on the (idle) DVE; one tiny fp32
                # matmul per q-sub does the final cross-partition reduction
                # (N=1 fp32 matmuls are slow on the PE, ~220ns each).
                acc = acc_pool.tile([P, QB], F32, tag="acc", name="acc")
                nc.vector.tensor_copy(acc, expS[:, 0:QB].bitcast(F32))
                for t in range(1, kt):
                    nc.vector.tensor_add(
                        acc, acc, expS[:, t * QB:(t + 1) * QB].bitcast(F32))
                den = pden.tile([P, 2], F32, tag="den", name="den")
                for s in range(2):
                    nc.tensor.matmul(den[:, s:s + 1],
                                     acc[:, P * s:P * (s + 1)], ones_f32,
                                     start=True, stop=True,
                                     skip_group_check=True)
                recip = recip_pool.tile([P, 2], F32, tag="recip", name="recip")
                nc.vector.reciprocal(recip, den)
                # pass 2: ctx[q, d] = sum_k expS[k, q] * V[k, d], then normalize
                for s in range(2):
                    for n in range(2):
                        ps = p512.tile([P, 512], F32, tag="mm512", name="ps_c")
                        for t in range(kt):
                            mm(ps, expS[:, t * QB + P * s: t * QB + P * (s + 1)],
                               V_sb[:, t * D + 512 * n: t * D + 512 * (n + 1)],
                               t == 0, t == kt - 1)
                        ot = out_pool.tile([P, 512], F32, tag="out", name="ot")
                        nc.vector.tensor_scalar_mul(ot, ps, recip[:, s:s + 1])
                        nc.sync.dma_start(
                            out=out_d[QB * jb + P * s: QB * jb + P * (s + 1),
                                      512 * n: 512 * (n + 1)],
                            in_=ot)


def build_nc():
    nc = bacc.Bacc("TRN2", target_bir_lowering=False, debug=False, num_devices=8)
    xT_d = nc.dram_tensor("xT", [D, T], BF16, kind="ExternalInput")
    xTk_d = nc.dram_tensor("xTk", [D, T // 2], BF16, kind="ExternalInput")
    xTq_d = nc.dram_tensor("xTq", [D, T // 2], BF16, kind="ExternalInput")
    wq_d = nc.dram_tensor("wq", [D, D], BF16, kind="ExternalInput")
    wk_d = nc.dram_tensor("wk", [D, D], BF16, kind="ExternalInput")
    wv_d = nc.dram_tensor("wv", [D, D], BF16, kind="ExternalInput")
    masks_d = nc.dram_tensor("masks", [4, P, QB], F32, kind="ExternalInput")
    out_d = nc.dram_tensor("out", [T // 2, D], F32, kind="ExternalOutput")
    with tile.TileContext(nc) as tc:
        _emit(nc, tc, xT_d[:], xTk_d[:], xTq_d[:], wq_d[:], wk_d[:], wv_d[:],
              masks_d[:],
              out_d[:])
    nc.compile()
    return nc


def make_masks(h):
    """Additive causal mask: 0 where key (128u + p) <= query (2j + h), else
    -1e9, within a 512-position diagonal window (positions relative to the
    q-block base).  Applied to raw scores before exp."""
    u = np.arange(4)[:, None, None]
    p = np.arange(P)[None, :, None]
    j = np.arange(QB)[None, None, :]
    vis = (128 * u + p <= 2 * j + h)
    return np.where(vis, 0.0, MASK_NEG).astype(np.float32)


def make_in_maps(x, W_query, W_key, W_value):
    wq = np.ascontiguousarray(W_query).astype(BF16_NP)
    wk = np.ascontiguousarray(W_key).astype(BF16_NP)
    wv = np.ascontiguousarray(W_value).astype(BF16_NP)
    masks = [make_masks(h) for h in range(2)]
    in_maps = []
    for core in range(8):
        b, h = divmod(core, 2)
        xb = np.asarray(x[b], dtype=np.float32)
        in_maps.append({
            "xT": np.ascontiguousarray(xb.T).astype(BF16_NP),
            "xTk": np.ascontiguousarray(xb[1024 * h:1024 * (h + 1)].T)
                   .astype(BF16_NP),
            "xTq": np.ascontiguousarray(xb[h::2].T).astype(BF16_NP),
            "wq": wq, "wk": wk, "wv": wv,
            "masks": masks[h],
        })
    return in_maps


_NC_CACHE = {}
LAST_EXEC_NS = None


def kernel(x, W_query, W_key, W_value):
    global LAST_EXEC_NS
    from concourse.bass_utils import run_bass_kernel_spmd

    if "nc" not in _NC_CACHE:
        _NC_CACHE["nc"] = build_nc()
    nc = _NC_CACHE["nc"]

    in_maps = make_in_maps(x, W_query, W_key, W_value)
    trace = bool(os.environ.get("BASS_TRACE"))
    res = run_bass_kernel_spmd(nc, in_maps, core_ids=list(range(8)), trace=trace)
    LAST_EXEC_NS = res.exec_time_ns

    out = np.empty((B, T, D), dtype=np.float32)
    for core in range(8):
        b, h = divmod(core, 2)
        out[b, h::2, :] = res.results[core]["out"]
    return out


if __name__ == "__main__":
    import time
    t0 = time.time()
    nc = build_nc()
    print(f"build+compile took {time.time() - t0:.1f}s")
    n_inst = sum(len(getattr(e, 'instructions', [])) for e in nc.engines) \
        if hasattr(nc, 'engines') else -1
    print("built ok")



# revision 3
# speedup vs baseline: 1.0966x; 1.0966x over previous
"""Causal single-head attention on 8 Trainium2 NeuronCores.

Problem: x[4, 2048, 1024] fp32, Wq/Wk/Wv[1024, 1024] fp32.
  q,k,v = x@Wq, x@Wk, x@Wv ; out = softmax(mask(q k^T)/32) @ v

Sharding (SPMD — one program, 8 cores, per-core data):
  core = 2*b + h  handles batch b, queries {t : t % 2 == h} (1024 queries).
  K^T AND V projections are split across the core pair by contiguous token
  half (core h projects tokens [1024h, 1024h+1024) from its xTk input) and
  exchanged with three pipelined pair-AllGathers that hide behind the
  remaining projections + early attention:
    G_K  (2 MB in): K^T own half (both 512-token windows)
    G_V1 (1 MB in): V own first window
    G_V2 (1 MB in): V own second window
  The program is h-independent: own projections go PSUM -> stage -> DRAM
  staging only, and K_sb/V_sb fill exclusively from gather outputs indexed
  by pair rank (rank r holds global token half r on every core).

Score pass is split by causal structure:
  - pass A (off-diagonal, fully unmasked): fp8 e4m3 DoubleRow matmuls
    (K=256 contraction per instruction, 2x bf16 FLOP rate; measured 110ns
    for K=256/N=256 vs 109ns for bf16 K=128/N=256).  k-tile t is needed by
    every q-block jb > t//4, so tile t is processed once against the
    merged query range [256*(t//4+1), 1024) in N<=512 groups.
  - pass B (diagonal, masked): bf16 at N=256 per (tile, q-block), additive
    mask then exp.  Keeping the diagonal bf16 bounds the fp8 noise:
    predicted rel err ~7e-3 vs the 2e-2 gate (all-fp8 scores would be
    1.4e-2; fp8 projections/AV measured 2.7e-2+ and are out).
  expS is stored bf16 for the whole own-query range and consumed per
  q-block by the AV pass (fully-masked diagonal tiles skipped for the
  first 128-query sub-block).

Head: the first K window runs chunk-outer with 4 concurrent PSUM groups x
2 waves so matmuls start as soon as (wk c0, x c0) land instead of after
the full 3 MB of input DMA.

Dtypes: bf16 matmul inputs except pass-A scores (fp8 e4m3, fp32 PSUM).
No max-subtraction in softmax (logits/32 ~ N(0, 0.41^2); exp never
overflows).  Denominator: DVE partition-partial sums + one tiny fp32r
ones-matmul per q-sub.
"""

import os
import numpy as np
import ml_dtypes

import concourse.mybir as mybir
import concourse.tile as tile
from concourse import bacc

F32 = mybir.dt.float32
F32R = mybir.dt.float32r
BF16 = mybir.dt.bfloat16
F8 = mybir.dt.float8e4
BF16_NP = ml_dtypes.bfloat16
F8_NP = ml_dtypes.float8_e4m3
DRM = mybir.MatmulPerfMode.DoubleRow

B, T, D = 4, 2048, 1024
P = 128
DC = D // P          # 8 contraction chunks
NT = T // P          # 16 key tiles
HT = T // 2          # own queries per core
QB = 256             # queries per q-block (per core)
NJB = HT // QB       # 4 q-blocks per core
SCALE = 1.0 / 32.0   # 1/sqrt(D)
MASK_NEG = -1.0e9
NT8 = 12             # k-tiles with an off-diagonal (fp8) part
PAIRS = [[0, 1], [2, 3], [4, 5], [6, 7]]
_EXP = mybir.ActivationFunctionType.Exp

# pass-A query groups per k-tile quarter (t//4): [(j0, n), ...]
A_GROUPS = {0: [(256, 512), (768, 256)], 1: [(512, 512)], 2: [(768, 256)]}


def _emit(nc, tc, xTk_d, xTq_d, wq_d, wk_d, wv_d, masks_d, out_d):
    def mm(out, lhsT, rhs, start, stop, **kw):
        if out.dtype == F32 and lhsT.dtype == F32:
            lhsT = lhsT.bitcast(F32R)
            rhs = rhs.bitcast(F32R)
        nc.tensor.matmul(out, lhsT, rhs, start=start, stop=stop, **kw)

    with (
        tc.sbuf_pool(name="persist", bufs=1) as persist,
        tc.psum_pool(name="p512", bufs=3) as p512,
    ):
        # persistent SBUF tensors
        K_sb = persist.tile([P, DC * T], BF16, tag="K", name="K_sb")
        K8 = persist.tile([P, DC, NT8 * P], F8, tag="K8", name="K8")
        V_sb = persist.tile([P, NT * D], BF16, tag="V", name="V_sb")
        Q_sb = persist.tile([P, DC * HT], BF16, tag="Q", name="Q_sb")
        Q8 = persist.tile([P, DC, HT], F8, tag="Q8", name="Q8")

        with (
            tc.sbuf_pool(name="wp", bufs=1) as wpool,
            tc.sbuf_pool(name="xp", bufs=1) as xpool,
            tc.sbuf_pool(name="stgp", bufs=6) as stg_pool,
            tc.psum_pool(name="pk8", bufs=4) as pk8,
            tc.tile_pool(name="drp", bufs=1, space="DRAM") as dr_pool,
            nc.named_scope("kv_proj"),
        ):
            wk_sb = wpool.tile([P, DC * D], BF16, tag="wk", name="wk_sb")
            wv_sb = wpool.tile([P, DC * D], BF16, tag="wv", name="wv_sb")
            xw = [xpool.tile([P, DC * 512], BF16, tag=f"xw{i}",
                             name=f"xw{i}") for i in range(2)]
            # interleave wk/x chunk DMAs so (wk c0, x c0) land first
            for c in range(DC):
                nc.sync.dma_start(out=wk_sb[:, c * D:(c + 1) * D],
                                  in_=wk_d[c * P:(c + 1) * P, :])
                nc.sync.dma_start(
                    out=xw[0][:, c * 512:(c + 1) * 512],
                    in_=xTk_d[c * P:(c + 1) * P, 0:512])
            for c in range(DC):
                nc.sync.dma_start(
                    out=xw[1][:, c * 512:(c + 1) * 512],
                    in_=xTk_d[c * P:(c + 1) * P, 512:1024])
                nc.sync.dma_start(out=wv_sb[:, c * D:(c + 1) * D],
                                  in_=wv_d[c * P:(c + 1) * P, :])

            kloc = dr_pool.tile([D, 1024], BF16, tag="kloc", name="kloc")
            kg = dr_pool.tile([2, D, 1024], BF16, tag="kg", name="kg")
            vloc = [dr_pool.tile([512, D], BF16, tag=f"vl{i}",
                                 name=f"vl{i}") for i in range(2)]
            vg = [dr_pool.tile([2, 512, D], BF16, tag=f"vg{i}",
                               name=f"vg{i}") for i in range(2)]

            def k_window(lw, c_outer):
                """K^T of own local window lw -> stage -> kloc columns."""
                pss = []
                if c_outer:
                    # chunk-outer, two waves of 4 concurrent PSUM groups:
                    # consume (wk c, x c) as the DMAs land
                    for wave in range(2):
                        pss_w = [pk8.tile([P, 512], F32, tag="mmk",
                                          name=f"psk{wave}{i}")
                                 for i in range(4)]
                        for c in range(DC):
                            for i, c2 in enumerate(range(4 * wave,
                                                         4 * wave + 4)):
                                mm(pss_w[i],
                                   wk_sb[:, c * D + P * c2:
                                         c * D + P * (c2 + 1)],
                                   xw[lw][:, c * 512:(c + 1) * 512],
                                   c == 0, c == DC - 1)
                        pss.extend(pss_w)
                else:
                    for c2 in range(DC):
                        ps = p512.tile([P, 512], F32, tag="mm512",
                                       name="ps_k")
                        for c in range(DC):
                            mm(ps, wk_sb[:, c * D + P * c2:
                                         c * D + P * (c2 + 1)],
                               xw[lw][:, c * 512:(c + 1) * 512],
                               c == 0, c == DC - 1)
                        pss.append(ps)
                for c2, ps in enumerate(pss):
                    st = stg_pool.tile([P, 512], BF16, tag="stk", name="stk")
                    nc.scalar.copy(out=st, in_=ps)
                    nc.sync.dma_start(
                        out=kloc[c2 * P:(c2 + 1) * P, 512 * lw:512 * (lw + 1)],
                        in_=st)

            def v_window(lw):
                for ts in range(4):
                    for n in range(2):
                        ps = p512.tile([P, 512], F32, tag="mm512",
                                       name="ps_v")
                        for c in range(DC):
                            mm(ps,
                               xw[lw][:, c * 512 + P * ts:
                                      c * 512 + P * (ts + 1)],
                               wv_sb[:, c * D + 512 * n: c * D + 512 * (n + 1)],
                               c == 0, c == DC - 1)
                        st = stg_pool.tile([P, 512], BF16, tag="stv",
                                           name="stv")
                        nc.scalar.copy(out=st, in_=ps)
                        nc.sync.dma_start(
                            out=vloc[lw][128 * ts:128 * (ts + 1),
                                         512 * n:512 * (n + 1)],
                            in_=st)

            def gather(ins, outs):
                nc.gpsimd.collective_compute(
                    "AllGather", mybir.AluOpType.bypass,
                    replica_groups=PAIRS, ins=[ins], outs=[outs])

            k_window(0, c_outer=True)
            k_window(1, c_outer=False)
            gather(kloc[:], kg[:])
            v_window(0)
            gather(vloc[0][:], vg[0][:])
            v_window(1)
            gather(vloc[1][:], vg[1][:])

            # unpack both halves by pair rank r (global token half r);
            # K first (needed at attention start), then V per window
            for r in range(2):
                for lw in range(2):
                    gw = 2 * r + lw
                    for c2 in range(DC):
                        dst = K_sb[:, c2 * T + 512 * gw:
                                   c2 * T + 512 * (gw + 1)]
                        nc.sync.dma_start(
                            out=dst,
                            in_=kg[r, c2 * P:(c2 + 1) * P,
                                   512 * lw:512 * (lw + 1)])
                        if gw < 3:  # fp8 cast for off-diagonal score tiles
                            nc.vector.tensor_copy(
                                K8[:, c2:c2 + 1, 512 * gw:512 * (gw + 1)],
                                dst)
            for lw in range(2):
                for r in range(2):
                    gw = 2 * r + lw
                    for ts in range(4):
                        t = 4 * gw + ts
                        nc.sync.dma_start(
                            out=V_sb[:, t * D:(t + 1) * D],
                            in_=vg[lw][r, 128 * ts:128 * (ts + 1), :])

            # Q^T projection (own interleaved queries)
            wq_sb = wpool.tile([P, DC * D], BF16, tag="wq", name="wq_sb")
            for c in range(DC):
                nc.sync.dma_start(out=wq_sb[:, c * D:(c + 1) * D],
                                  in_=wq_d[c * P:(c + 1) * P, :])
            xtqs = []
            for jp in range(2):
                xtq = xpool.tile([P, DC * 512], BF16, tag=f"xq{jp}",
                                 name=f"xtq{jp}")
                for c in range(DC):
                    nc.sync.dma_start(
                        out=xtq[:, c * 512:(c + 1) * 512],
                        in_=xTq_d[c * P:(c + 1) * P, 512 * jp:512 * (jp + 1)])
                xtqs.append(xtq)
            with nc.named_scope("q_proj"):
                for jp in range(2):
                    for c2 in range(DC):
                        ps = p512.tile([P, 512], F32, tag="mm512",
                                       name="ps_q")
                        for c in range(DC):
                            mm(ps,
                               wq_sb[:, c * D + P * c2: c * D + P * (c2 + 1)],
                               xtqs[jp][:, c * 512:(c + 1) * 512],
                               c == 0, c == DC - 1)
                        nc.scalar.copy(
                            out=Q_sb[:, c2 * HT + 512 * jp:
                                     c2 * HT + 512 * (jp + 1)],
                            in_=ps)
                        nc.vector.tensor_copy(
                            Q8[:, c2:c2 + 1, 512 * jp:512 * (jp + 1)], ps)

        # ---- attention ----
        with (
            tc.sbuf_pool(name="attnp", bufs=1) as attnp,
            tc.sbuf_pool(name="recipp", bufs=2) as recip_pool,
            tc.sbuf_pool(name="accp", bufs=2) as acc_pool,
            tc.sbuf_pool(name="outp", bufs=4) as out_pool,
            tc.psum_pool(name="p256", bufs=4) as p256,
            tc.psum_pool(name="pden", bufs=1) as pden,
            nc.named_scope("attn"),
        ):
            expS = attnp.tile([P, NT * HT], BF16, tag="E", name="expS")
            mask_sb = attnp.tile([P, 4 * QB], F32, tag="M", name="mask_sb")
            ones_f32 = attnp.tile([P, 1], F32, tag="O32", name="ones_f32")
            nc.vector.memset(ones_f32, 1.0)
            for u in range(4):
                nc.sync.dma_start(out=mask_sb[:, u * QB:(u + 1) * QB],
                                  in_=masks_d[u])

            def pass_a(tiles):
                # off-diagonal scores: fp8 DoubleRow, merged query ranges
                for t in tiles:
                    for j0, n in A_GROUPS[t // 4]:
                        psp = p512 if n == 512 else p256
                        ps = psp.tile([P, n], F32,
                                      tag="mm512" if n == 512 else "mm256",
                                      name="ps_a")
                        for cp in range(DC // 2):
                            nc.tensor.matmul(
                                ps,
                                K8[:, 2 * cp:2 * cp + 2, P * t:P * (t + 1)],
                                Q8[:, 2 * cp:2 * cp + 2, j0:j0 + n],
                                start=cp == 0, stop=cp == DC // 2 - 1,
                                perf_mode=DRM)
                        nc.scalar.activation(
                            out=expS[:, t * HT + j0: t * HT + j0 + n],
                            in_=ps, func=_EXP, scale=SCALE)

            def pass_b(jb):
                # diagonal scores: bf16 + additive mask
                for u in range(4):
                    t = 4 * jb + u
                    ps = p256.tile([P, QB], F32, tag="mm256", name="ps_b")
                    for c in range(DC):
                        mm(ps, K_sb[:, c * T + P * t: c * T + P * (t + 1)],
                           Q_sb[:, c * HT + QB * jb: c * HT + QB * (jb + 1)],
                           c == 0, c == DC - 1)
                    nc.vector.tensor_add(ps, ps,
                                         mask_sb[:, u * QB:(u + 1) * QB])
                    nc.scalar.activation(
                        out=expS[:, t * HT + QB * jb: t * HT + QB * (jb + 1)],
                        in_=ps, func=_EXP, scale=SCALE)

            def av(jb):
                kt = 4 * (jb + 1)
                e_col = lambda t: expS[:, t * HT + QB * jb:
                                       t * HT + QB * (jb + 1)]
                acc = acc_pool.tile([P, QB], F32, tag="acc", name="acc")
                nc.vector.tensor_copy(acc, e_col(0))
                for t in range(1, kt):
                    nc.vector.tensor_add(acc, acc, e_col(t))
                den = pden.tile([P, 2], F32, tag="den", name="den")
                for s in range(2):
                    nc.tensor.matmul(den[:, s:s + 1],
                                     acc[:, P * s:P * (s + 1)], ones_f32,
                                     start=True, stop=True,
                                     skip_group_check=True)
                recip = recip_pool.tile([P, 2], F32, tag="recip",
                                        name="recip")
                nc.vector.reciprocal(recip, den)
                for s in range(2):
                    for n in range(2):
                        ps = p512.tile([P, 512], F32, tag="mm512",
                                       name="ps_c")
                        ts_av = [t for t in range(kt)
                                 if not (s == 0 and t >= kt - 2)]
                        for i, t in enumerate(ts_av):
                            mm(ps, expS[:, t * HT + QB * jb + P * s:
                                        t * HT + QB * jb + P * (s + 1)],
                               V_sb[:, t * D + 512 * n: t * D + 512 * (n + 1)],
                               i == 0, i == len(ts_av) - 1)
                        ot = out_pool.tile([P, 512], F32, tag="out",
                                           name="ot")
                        nc.vector.tensor_scalar_mul(ot, ps, recip[:, s:s + 1])
                        nc.sync.dma_start(
                            out=out_d[QB * jb + P * s: QB * jb + P * (s + 1),
                                      512 * n: 512 * (n + 1)],
                            in_=ot)

            pass_a([0, 1, 2, 3])
            pass_b(0)
            av(0)
            pass_a([4, 5, 6, 7])
            pass_b(1)
            av(1)
            pass_a([8, 9, 10, 11])
            pass_b(2)
            av(2)
            pass_b(3)
            av(3)


def build_nc():
    nc = bacc.Bacc("TRN2", target_bir_lowering=False, debug=False,
                   num_devices=8)
    xTk_d = nc.dram_tensor("xTk", [D, T // 2], BF16, kind="ExternalInput")
    xTq_d = nc.dram_tensor("xTq", [D, T // 2], BF16, kind="ExternalInput")
    wq_d = nc.dram_tensor("wq", [D, D], BF16, kind="ExternalInput")
    wk_d = nc.dram_tensor("wk", [D, D], BF16, kind="ExternalInput")
    wv_d = nc.dram_tensor("wv", [D, D], BF16, kind="ExternalInput")
    masks_d = nc.dram_tensor("masks", [4, P, QB], F32, kind="ExternalInput")
    out_d = nc.dram_tensor("out", [T // 2, D], F32, kind="ExternalOutput")
    with tile.TileContext(nc) as tc:
        _emit(nc, tc, xTk_d[:], xTq_d[:], wq_d[:], wk_d[:], wv_d[:],
              masks_d[:], out_d[:])
    nc.compile()
    return nc


def make_masks(h):
    """Additive causal mask: 0 where key (128u + p) <= query (2j + h), else
    -1e9, within a 512-position diagonal window (positions relative to the
    q-block base).  Applied to raw scores before exp."""
    u = np.arange(4)[:, None, None]
    p = np.arange(P)[None, :, None]
    j = np.arange(QB)[None, None, :]
    vis = (128 * u + p <= 2 * j + h)
    return np.where(vis, 0.0, MASK_NEG).astype(np.float32)


def make_in_maps(x, W_query, W_key, W_value):
    wq = np.ascontiguousarray(W_query).astype(BF16_NP)
    wk = np.ascontiguousarray(W_key).astype(BF16_NP)
    wv = np.ascontiguousarray(W_value).astype(BF16_NP)
    masks = [make_masks(h) for h in range(2)]
    in_maps = []
    for core in range(8):
        b, h = divmod(core, 2)
        xb = np.asarray(x[b], dtype=np.float32)
        in_maps.append({
            "xTk": np.ascontiguousarray(xb[1024 * h:1024 * (h + 1)].T)
                   .astype(BF16_NP),
            "xTq": np.ascontiguousarray(xb[h::2].T).astype(BF16_NP),
            "wq": wq, "wk": wk, "wv": wv,
            "masks": masks[h],
        })
    return in_maps


_NC_CACHE = {}
LAST_EXEC_NS = None


def kernel(x, W_query, W_key, W_value):
    global LAST_EXEC_NS
    from concourse.bass_utils import run_bass_kernel_spmd

    if "nc" not in _NC_CACHE:
        _NC_CACHE["nc"] = build_nc()
    nc = _NC_CACHE["nc"]

    in_maps = make_in_maps(x, W_query, W_key, W_value)
    trace = bool(os.environ.get("BASS_TRACE"))
    res = run_bass_kernel_spmd(nc, in_maps, core_ids=list(range(8)),
                               trace=trace)
    LAST_EXEC_NS = res.exec_time_ns

    out = np.empty((B, T, D), dtype=np.float32)
    for core in range(8):
        b, h = divmod(core, 2)
        out[b, h::2, :] = res.results[core]["out"]
    return out


if __name__ == "__main__":
    import time
    t0 = time.time()
    nc = build_nc()
    print(f"build+compile took {time.time() - t0:.1f}s")
    print("built ok")


# revision 13
# speedup vs baseline: 1.1090x; 1.0113x over previous
"""Causal single-head attention on 8 Trainium2 NeuronCores.

Problem: x[4, 2048, 1024] fp32, Wq/Wk/Wv[1024, 1024] fp32.
  q,k,v = x@Wq, x@Wk, x@Wv ; out = softmax(mask(q k^T)/32) @ v

Sharding (SPMD — one program, 8 cores, per-core data):
  core = 2*b + h  handles batch b, queries {t : t % 2 == h} (1024 queries).
  K^T AND V projections are split across the core pair by contiguous token
  half (core h projects tokens [1024h, 1024h+1024) from its xTk input) and
  exchanged with three pipelined pair-AllGathers that hide behind the
  remaining projections + early attention:
    G_K  (2 MB in): K^T own half (both 512-token windows)
    G_V1 (1 MB in): V own first window
    G_V2 (1 MB in): V own second window
  The program is h-independent: own projections go PSUM -> stage -> DRAM
  staging only, and K_sb/V_sb fill exclusively from gather outputs indexed
  by pair rank (rank r holds global token half r on every core).

Score pass is split by causal structure:
  - pass A (off-diagonal, fully unmasked): fp8 e4m3 DoubleRow matmuls
    (K=256 contraction per instruction, 2x bf16 FLOP rate; measured 110ns
    for K=256/N=256 vs 109ns for bf16 K=128/N=256).  k-tile t is needed by
    every q-block jb > t//4, so tile t is processed once against the
    merged query range [256*(t//4+1), 1024) in N<=512 groups.
  - pass B (diagonal, masked): bf16 at N=256 per (tile, q-block), additive
    mask then exp.  Keeping the diagonal bf16 bounds the fp8 noise:
    predicted rel err ~7e-3 vs the 2e-2 gate (all-fp8 scores would be
    1.4e-2; fp8 projections/AV measured 2.7e-2+ and are out).
  expS is stored bf16 for the whole own-query range and consumed per
  q-block by the AV pass (fully-masked diagonal tiles skipped for the
  first 128-query sub-block).

Head: the first K window runs chunk-outer with 4 concurrent PSUM groups x
2 waves so matmuls start as soon as (wk c0, x c0) land instead of after
the full 3 MB of input DMA.

Dtypes: bf16 matmul inputs except pass-A scores (fp8 e4m3, fp32 PSUM).
No max-subtraction in softmax (logits/32 ~ N(0, 0.41^2); exp never
overflows).  Denominator: DVE partition-partial sums + one tiny fp32r
ones-matmul per q-sub.
"""

import os
import numpy as np
import ml_dtypes

import concourse.mybir as mybir
import concourse.tile as tile
from concourse import bacc

F32 = mybir.dt.float32
F32R = mybir.dt.float32r
BF16 = mybir.dt.bfloat16
F8 = mybir.dt.float8e4
BF16_NP = ml_dtypes.bfloat16
F8_NP = ml_dtypes.float8_e4m3
DRM = mybir.MatmulPerfMode.DoubleRow

B, T, D = 4, 2048, 1024
P = 128
DC = D // P          # 8 contraction chunks
NT = T // P          # 16 key tiles
HT = T // 2          # own queries per core
QB = 256             # queries per q-block (per core)
NJB = HT // QB       # 4 q-blocks per core
SCALE = 1.0 / 32.0   # 1/sqrt(D)
MASK_NEG = -1.0e9
NT8 = 12             # k-tiles with an off-diagonal (fp8) part
PAIRS = [[0, 1], [2, 3], [4, 5], [6, 7]]
_EXP = mybir.ActivationFunctionType.Exp

# pass-A query groups per k-tile quarter (t//4): [(j0, n), ...]
A_GROUPS = {0: [(256, 512), (768, 256)], 1: [(512, 512)], 2: [(768, 256)]}


def _emit(nc, tc, xTk_d, xTq_d, wq_d, wk_d, wv_d, masks_d, out_d):
    def mm(out, lhsT, rhs, start, stop, **kw):
        if out.dtype == F32 and lhsT.dtype == F32:
            lhsT = lhsT.bitcast(F32R)
            rhs = rhs.bitcast(F32R)
        nc.tensor.matmul(out, lhsT, rhs, start=start, stop=stop, **kw)

    with (
        tc.sbuf_pool(name="persist", bufs=1) as persist,
        tc.psum_pool(name="p512", bufs=3) as p512,
        tc.tile_pool(name="drp", bufs=1, space="DRAM") as dr_pool,
    ):
        # persistent SBUF tensors
        K_sb = persist.tile([P, DC * T], BF16, tag="K", name="K_sb")
        K8 = persist.tile([P, DC, NT8 * P], F8, tag="K8", name="K8")
        V_sb = persist.tile([P, NT * D], BF16, tag="V", name="V_sb")
        Q_sb = persist.tile([P, DC * HT], BF16, tag="Q", name="Q_sb")
        Q8 = persist.tile([P, DC, HT], F8, tag="Q8", name="Q8")
        mask_sb = persist.tile([P, 4 * QB], F32, tag="M", name="mask_sb")

        with (
            tc.sbuf_pool(name="wp", bufs=1) as wpool,
            tc.sbuf_pool(name="xp", bufs=1) as xpool,
            tc.sbuf_pool(name="stgp", bufs=6) as stg_pool,
            tc.psum_pool(name="pk8", bufs=4) as pk8,
            nc.named_scope("kv_proj"),
        ):
            wk_sb = wpool.tile([P, DC * D], BF16, tag="wk", name="wk_sb")
            wv_sb = wpool.tile([P, DC * D], BF16, tag="wv", name="wv_sb")
            xw = [xpool.tile([P, DC * 512], BF16, tag=f"xw{i}",
                             name=f"xw{i}") for i in range(2)]
            # interleave wk/x chunk DMAs so (wk c0, x c0) land first
            for c in range(DC):
                nc.sync.dma_start(out=wk_sb[:, c * D:(c + 1) * D],
                                  in_=wk_d[c * P:(c + 1) * P, :])
                nc.sync.dma_start(
                    out=xw[0][:, c * 512:(c + 1) * 512],
                    in_=xTk_d[c * P:(c + 1) * P, 0:512])
            for c in range(DC):
                nc.sync.dma_start(
                    out=xw[1][:, c * 512:(c + 1) * 512],
                    in_=xTk_d[c * P:(c + 1) * P, 512:1024])
                nc.sync.dma_start(out=wv_sb[:, c * D:(c + 1) * D],
                                  in_=wv_d[c * P:(c + 1) * P, :])

            kloc = [dr_pool.tile([D, 512], BF16, tag=f"kl{i}",
                                 name=f"kl{i}") for i in range(2)]
            kg = [dr_pool.tile([2, D, 512], BF16, tag=f"kg{i}",
                               name=f"kg{i}") for i in range(2)]
            vloc = [dr_pool.tile([512, D], BF16, tag=f"vl{i}",
                                 name=f"vl{i}") for i in range(2)]
            vg = [dr_pool.tile([2, 512, D], BF16, tag=f"vg{i}",
                               name=f"vg{i}") for i in range(2)]

            def k_window(lw, c_outer):
                """K^T of own local window lw -> stage -> kloc columns."""
                pss = []
                if c_outer:
                    # chunk-outer, two waves of 4 concurrent PSUM groups:
                    # consume (wk c, x c) as the DMAs land
                    for wave in range(2):
                        pss_w = [pk8.tile([P, 512], F32, tag="mmk",
                                          name=f"psk{wave}{i}")
                                 for i in range(4)]
                        for c in range(DC):
                            for i, c2 in enumerate(range(4 * wave,
                                                         4 * wave + 4)):
                                mm(pss_w[i],
                                   wk_sb[:, c * D + P * c2:
                                         c * D + P * (c2 + 1)],
                                   xw[lw][:, c * 512:(c + 1) * 512],
                                   c == 0, c == DC - 1)
                        pss.extend(pss_w)
                else:
                    for c2 in range(DC):
                        ps = p512.tile([P, 512], F32, tag="mm512",
                                       name="ps_k")
                        for c in range(DC):
                            mm(ps, wk_sb[:, c * D + P * c2:
                                         c * D + P * (c2 + 1)],
                               xw[lw][:, c * 512:(c + 1) * 512],
                               c == 0, c == DC - 1)
                        pss.append(ps)
                for c2, ps in enumerate(pss):
                    st = stg_pool.tile([P, 512], BF16, tag="stk", name="stk")
                    nc.scalar.copy(out=st, in_=ps)
                    nc.sync.dma_start(
                        out=kloc[lw][c2 * P:(c2 + 1) * P, :], in_=st)

            def v_window(lw):
                for ts in range(4):
                    for n in range(2):
                        ps = p512.tile([P, 512], F32, tag="mm512",
                                       name="ps_v")
                        for c in range(DC):
                            mm(ps,
                               xw[lw][:, c * 512 + P * ts:
                                      c * 512 + P * (ts + 1)],
                               wv_sb[:, c * D + 512 * n: c * D + 512 * (n + 1)],
                               c == 0, c == DC - 1)
                        st = stg_pool.tile([P, 512], BF16, tag="stv",
                                           name="stv")
                        nc.scalar.copy(out=st, in_=ps)
                        nc.sync.dma_start(
                            out=vloc[lw][128 * ts:128 * (ts + 1),
                                         512 * n:512 * (n + 1)],
                            in_=st)

            def gather(ins, outs):
                nc.gpsimd.collective_compute(
                    "AllGather", mybir.AluOpType.bypass,
                    replica_groups=PAIRS, ins=[ins], outs=[outs])

            k_window(0, c_outer=True)
            gather(kloc[0][:], kg[0][:])
            k_window(1, c_outer=False)
            gather(kloc[1][:], kg[1][:])
            v_window(0)
            gather(vloc[0][:], vg[0][:])
            v_window(1)
            gather(vloc[1][:], vg[1][:])

            # Q-proj inputs + masks BEFORE the gather-dependent unpack DMAs
            # (the Sync queue issues in order; an unpack DMA waiting on a
            # gather semaphore would head-of-line block these otherwise)
            wq_sb = wpool.tile([P, DC * D], BF16, tag="wq", name="wq_sb")
            for c in range(DC):
                nc.sync.dma_start(out=wq_sb[:, c * D:(c + 1) * D],
                                  in_=wq_d[c * P:(c + 1) * P, :])
            xtqs = []
            for jp in range(2):
                xtq = xpool.tile([P, DC * 512], BF16, tag=f"xq{jp}",
                                 name=f"xtq{jp}")
                for c in range(DC):
                    nc.sync.dma_start(
                        out=xtq[:, c * 512:(c + 1) * 512],
                        in_=xTq_d[c * P:(c + 1) * P, 512 * jp:512 * (jp + 1)])
                xtqs.append(xtq)
            for u in range(4):
                nc.sync.dma_start(out=mask_sb[:, u * QB:(u + 1) * QB],
                                  in_=masks_d[u])

            # unpack by pair rank r (global token half r): K both windows
            # (needed at attention start), then V first-window pair {w0,w2};
            # the {w1,w3} pair is unpacked inside the attention phase so its
            # wait on G_V2 doesn't block earlier Sync traffic
            for lw in range(2):
                for r in range(2):
                    gw = 2 * r + lw
                    for c2 in range(DC):
                        dst = K_sb[:, c2 * T + 512 * gw:
                                   c2 * T + 512 * (gw + 1)]
                        nc.sync.dma_start(
                            out=dst,
                            in_=kg[lw][r, c2 * P:(c2 + 1) * P, :])
                        if gw < 3:  # fp8 cast for off-diagonal score tiles
                            nc.vector.tensor_copy(
                                K8[:, c2:c2 + 1, 512 * gw:512 * (gw + 1)],
                                dst)
            for r in range(2):
                gw = 2 * r
                for ts in range(4):
                    t = 4 * gw + ts
                    nc.sync.dma_start(
                        out=V_sb[:, t * D:(t + 1) * D],
                        in_=vg[0][r, 128 * ts:128 * (ts + 1), :])

            with nc.named_scope("q_proj"):
                for jp in range(2):
                    for c2 in range(DC):
                        ps = p512.tile([P, 512], F32, tag="mm512",
                                       name="ps_q")
                        for c in range(DC):
                            mm(ps,
                               wq_sb[:, c * D + P * c2: c * D + P * (c2 + 1)],
                               xtqs[jp][:, c * 512:(c + 1) * 512],
                               c == 0, c == DC - 1)
                        nc.scalar.copy(
                            out=Q_sb[:, c2 * HT + 512 * jp:
                                     c2 * HT + 512 * (jp + 1)],
                            in_=ps)
                        nc.vector.tensor_copy(
                            Q8[:, c2:c2 + 1, 512 * jp:512 * (jp + 1)], ps)

        # ---- attention ----
        with (
            tc.sbuf_pool(name="attnp", bufs=1) as attnp,
            tc.sbuf_pool(name="recipp", bufs=2) as recip_pool,
            tc.sbuf_pool(name="accp", bufs=1) as acc_pool,
            tc.sbuf_pool(name="outp", bufs=4) as out_pool,
            tc.psum_pool(name="p256", bufs=4) as p256,
            tc.psum_pool(name="pden", bufs=1) as pden,
            nc.named_scope("attn"),
        ):
            expS = attnp.tile([P, NT * HT], BF16, tag="E", name="expS")
            ones_f32 = attnp.tile([P, 1], F32, tag="O32", name="ones_f32")
            nc.vector.memset(ones_f32, 1.0)
            # per-q-block denominator partial sums (DVE), accumulated as
            # expS tiles land so the den matmuls never wait on a long chain
            accs = [acc_pool.tile([P, QB], F32, tag=f"acc{jb}",
                                  name=f"acc{jb}") for jb in range(NJB)]
            acc_first = [True] * NJB

            def acc_add(t, jb):
                e_col = expS[:, t * HT + QB * jb: t * HT + QB * (jb + 1)]
                if acc_first[jb]:
                    nc.vector.tensor_copy(accs[jb], e_col)
                    acc_first[jb] = False
                else:
                    nc.vector.tensor_add(accs[jb], accs[jb], e_col)

            def v_unpack_late():
                # peer/own V windows {w1, w3} from G_V2, emitted here so
                # their gather wait doesn't stall earlier Sync traffic
                for r in range(2):
                    gw = 2 * r + 1
                    for ts in range(4):
                        t = 4 * gw + ts
                        nc.sync.dma_start(
                            out=V_sb[:, t * D:(t + 1) * D],
                            in_=vg[1][r, 128 * ts:128 * (ts + 1), :])

            def pass_a(tiles):
                # off-diagonal scores: fp8 DoubleRow, merged query ranges
                for t in tiles:
                    for j0, n in A_GROUPS[t // 4]:
                        psp = p512 if n == 512 else p256
                        ps = psp.tile([P, n], F32,
                                      tag="mm512" if n == 512 else "mm256",
                                      name="ps_a")
                        for cp in range(DC // 2):
                            nc.tensor.matmul(
                                ps,
                                K8[:, 2 * cp:2 * cp + 2, P * t:P * (t + 1)],
                                Q8[:, 2 * cp:2 * cp + 2, j0:j0 + n],
                                start=cp == 0, stop=cp == DC // 2 - 1,
                                perf_mode=DRM)
                        nc.scalar.activation(
                            out=expS[:, t * HT + j0: t * HT + j0 + n],
                            in_=ps, func=_EXP, scale=SCALE)
                    for jb in range(t // 4 + 1, NJB):
                        acc_add(t, jb)

            def pass_b(jb):
                # diagonal scores: bf16 + additive mask
                for u in range(4):
                    t = 4 * jb + u
                    ps = p256.tile([P, QB], F32, tag="mm256", name="ps_b")
                    for c in range(DC):
                        mm(ps, K_sb[:, c * T + P * t: c * T + P * (t + 1)],
                           Q_sb[:, c * HT + QB * jb: c * HT + QB * (jb + 1)],
                           c == 0, c == DC - 1)
                    nc.vector.tensor_add(ps, ps,
                                         mask_sb[:, u * QB:(u + 1) * QB])
                    nc.scalar.activation(
                        out=expS[:, t * HT + QB * jb: t * HT + QB * (jb + 1)],
                        in_=ps, func=_EXP, scale=SCALE)
                    acc_add(t, jb)

            def av(jb):
                kt = 4 * (jb + 1)
                den = pden.tile([P, 2], F32, tag="den", name="den")
                for s in range(2):
                    nc.tensor.matmul(den[:, s:s + 1],
                                     accs[jb][:, P * s:P * (s + 1)], ones_f32,
                                     start=True, stop=True,
                                     skip_group_check=True)
                recip = recip_pool.tile([P, 2], F32, tag="recip",
                                        name="recip")
                nc.vector.reciprocal(recip, den)
                for s in range(2):
                    for n in range(2):
                        ps = p512.tile([P, 512], F32, tag="mm512",
                                       name="ps_c")
                        ts_av = [t for t in range(kt)
                                 if not (s == 0 and t >= kt - 2)]
                        for i, t in enumerate(ts_av):
                            mm(ps, expS[:, t * HT + QB * jb + P * s:
                                        t * HT + QB * jb + P * (s + 1)],
                               V_sb[:, t * D + 512 * n: t * D + 512 * (n + 1)],
                               i == 0, i == len(ts_av) - 1)
                        ot = out_pool.tile([P, 512], F32, tag="out",
                                           name="ot")
                        nc.vector.tensor_scalar_mul(ot, ps, recip[:, s:s + 1])
                        nc.sync.dma_start(
                            out=out_d[QB * jb + P * s: QB * jb + P * (s + 1),
                                      512 * n: 512 * (n + 1)],
                            in_=ot)

            pass_a([0, 1, 2, 3])
            pass_b(0)
            v_unpack_late()
            av(0)
            pass_a([4, 5, 6, 7])
            pass_b(1)
            av(1)
            pass_a([8, 9, 10, 11])
            pass_b(2)
            av(2)
            pass_b(3)
            av(3)


def build_nc():
    nc = bacc.Bacc("TRN2", target_bir_lowering=False, debug=False,
                   num_devices=8)
    xTk_d = nc.dram_tensor("xTk", [D, T // 2], BF16, kind="ExternalInput")
    xTq_d = nc.dram_tensor("xTq", [D, T // 2], BF16, kind="ExternalInput")
    wq_d = nc.dram_tensor("wq", [D, D], BF16, kind="ExternalInput")
    wk_d = nc.dram_tensor("wk", [D, D], BF16, kind="ExternalInput")
    wv_d = nc.dram_tensor("wv", [D, D], BF16, kind="ExternalInput")
    masks_d = nc.dram_tensor("masks", [4, P, QB], F32, kind="ExternalInput")
    out_d = nc.dram_tensor("out", [T // 2, D], F32, kind="ExternalOutput")
    with tile.TileContext(nc) as tc:
        _emit(nc, tc, xTk_d[:], xTq_d[:], wq_d[:], wk_d[:], wv_d[:],
              masks_d[:], out_d[:])
    nc.compile()
    return nc


def make_masks(h):
    """Additive causal mask: 0 where key (128u + p) <= query (2j + h), else
    -1e9, within a 512-position diagonal window (positions relative to the
    q-block base).  Applied to raw scores before exp."""
    u = np.arange(4)[:, None, None]
    p = np.arange(P)[None, :, None]
    j = np.arange(QB)[None, None, :]
    vis = (128 * u + p <= 2 * j + h)
    return np.where(vis, 0.0, MASK_NEG).astype(np.float32)


def make_in_maps(x, W_query, W_key, W_value):
    wq = np.ascontiguousarray(W_query).astype(BF16_NP)
    wk = np.ascontiguousarray(W_key).astype(BF16_NP)
    wv = np.ascontiguousarray(W_value).astype(BF16_NP)
    masks = [make_masks(h) for h in range(2)]
    in_maps = []
    for core in range(8):
        b, h = divmod(core, 2)
        xb = np.asarray(x[b], dtype=np.float32)
        in_maps.append({
            "xTk": np.ascontiguousarray(xb[1024 * h:1024 * (h + 1)].T)
                   .astype(BF16_NP),
            "xTq": np.ascontiguousarray(xb[h::2].T).astype(BF16_NP),
            "wq": wq, "wk": wk, "wv": wv,
            "masks": masks[h],
        })
    return in_maps


_NC_CACHE = {}
LAST_EXEC_NS = None


def kernel(x, W_query, W_key, W_value):
    global LAST_EXEC_NS
    from concourse.bass_utils import run_bass_kernel_spmd

    if "nc" not in _NC_CACHE:
        _NC_CACHE["nc"] = build_nc()
    nc = _NC_CACHE["nc"]

    in_maps = make_in_maps(x, W_query, W_key, W_value)
    trace = bool(os.environ.get("BASS_TRACE"))
    res = run_bass_kernel_spmd(nc, in_maps, core_ids=list(range(8)),
                               trace=trace)
    LAST_EXEC_NS = res.exec_time_ns

    out = np.empty((B, T, D), dtype=np.float32)
    for core in range(8):
        b, h = divmod(core, 2)
        out[b, h::2, :] = res.results[core]["out"]
    return out


if __name__ == "__main__":
    import time
    t0 = time.time()
    nc = build_nc()
    print(f"build+compile took {time.time() - t0:.1f}s")
    print("built ok")
